# revision 5
# baseline (speedup 1.0000x reference)
"""CapsLayer2D dynamic-routing kernel for 8x TRN2 NeuronCores — v2.

Shapes (hardcoded):
  inputs: [B=16, R=8, C=8, I=128, DIN=16] fp32
  W:      [K=32, I=128, DIN=16, DOUT=16] fp32
  out:    [B, R, C, K, DOUT] fp32

Math: 3-round dynamic routing. Closed form (verified 6e-6 vs reference):
  U[p,k] = res (I x O);  s0 = mean_i U_i;  A = U^T U
  y1 = A s0 ; y2 = A y1
  g = factor(s0); s1 = s0 + g*y1; f = factor(s1)
  out = factor(s2)*s2,  s2 = s0 + (g+f)*y1 + f*g*y2
  factor(s) = (|s|^2/(1+|s|^2)) / sqrt(|s|^2+eps)
y1 = U^T(U s0), y2 = U^T(U y1): each is one uv pass (contract o) and one
ut pass (contract i) over res — four DVE sweeps total, all in 2x mode.

Sharding: batch across 8 cores (128 positions/core), W replicated.

Per-core plan:
  Host pre-builds Xt [(i%4)*32+d, chunk*128+p] and W_r [(i%4)*32+d,
  chunk*512 + k*16+o] fp16 (d padded 16->32). PE: 32 accumulating
  matmuls -> s0; 512 row-banded 128-col matmuls -> res fp16 in
  (g, i, k8, o) order (g = k-group of 8). Scalar engine evacuates PSUM.
  Routing per (round, group): uv-mul (src1 bcast over i, 2x), o-tree
  adds 16->8->4->2 (2x) + q2-direct (dup'd pair layout), ut-mul via
  (oh=8, ol=2) pair views (2x), i-tree 7 halving adds (2x).
  Squash factors g/f/h on Scalar/Vector smalls off the sweep path.
"""

import sys

import numpy as np

sys.path.insert(0, "/opt/trn_rl_repo")

P, I, D, D2, K, O = 128, 128, 16, 32, 32, 16
KC = 8          # k-group size
NG = K // KC    # 4 groups
GN = I * KC * O  # 16384 elements per group block
KO = K * O      # 512
N_CORES = 8
EPS = 1e-7

_PROGRAM = None


def _build_program():
    from contextlib import ExitStack

    import concourse.tile as tile
    from concourse import bacc, mybir

    F32 = mybir.dt.float32
    F16 = mybir.dt.float16
    ADD = mybir.AluOpType.add
    X = mybir.AxisListType.X
    SQRT = mybir.ActivationFunctionType.Sqrt

    nc = bacc.Bacc("TRN2", target_bir_lowering=False, debug=False)

    xt_d = nc.dram_tensor("xt", [P, 32 * 128], F16, kind="ExternalInput").ap()
    wr_d = nc.dram_tensor("wr", [P, 32 * KO], F16, kind="ExternalInput").ap()
    out_d = nc.dram_tensor("out", [P, KO], F32, kind="ExternalOutput").ap()

    with ExitStack() as ctx:
        tc = ctx.enter_context(tile.TileContext(nc))

        pp = ctx.enter_context(tc.tile_pool(name="pp", bufs=2, space="PSUM"))
        rp = ctx.enter_context(tc.tile_pool(name="resp", bufs=1))
        sm = ctx.enter_context(tc.tile_pool(name="small", bufs=1))

        res = rp.tile([P, NG * GN], F16)  # [P, 65536]

        # ---- small tiles ----
        s0f = sm.tile([P, KO], F32, tag="s0f")
        s0h = sm.tile([P, KO], F16, tag="s0h")
        y1h = sm.tile([P, KO], F16, tag="y1h")
        y2h = sm.tile([P, KO], F16, tag="y2h")
        sqb = sm.tile([P, KO], F32, tag="sqb")
        s2f = sm.tile([P, KO], F32, tag="s2f")
        eps_t = sm.tile([P, 1], F32, tag="eps")
        nc.vector.memset(eps_t[:], EPS)

        def factor(src_f32, out32, tag):
            """out32[p, K] = (nsq/(1+nsq))/sqrt(nsq+eps), nsq over o."""
            nc.scalar.square(sqb[:], src_f32)
            nsq = sm.tile([P, K], F32, tag=f"nsq_{tag}")
            nc.vector.tensor_reduce(
                nsq[:], sqb[:].rearrange("p (k o) -> p k o", k=K), X, ADD
            )
            rt = sm.tile([P, K], F32, tag="f_rt")
            nc.scalar.activation(rt[:], nsq[:], SQRT, bias=eps_t[:])
            b1 = sm.tile([P, K], F32, tag="f_b1")
            nc.scalar.add(b1[:], nsq[:], 1.0)
            den = sm.tile([P, K], F32, tag="f_den")
            nc.vector.tensor_mul(den[:], rt[:], b1[:])
            rin = sm.tile([P, K], F32, tag="f_rin")
            nc.vector.reciprocal(rin[:], den[:])
            nc.vector.tensor_mul(out32[:], nsq[:], rin[:])

        def bcast_o(v32):
            return (
                v32[:]
                .rearrange("p k -> p k", k=K)
                .unsqueeze(2)
                .broadcast_to([P, K, O])
            )

        with tc.tile_pool(name="prep", bufs=1) as prep:
            Xt = prep.tile([P, 32 * 128], F16, tag="xt")
            Wr = prep.tile([P, 32 * KO], F16, tag="wr")
            for q in range(2):
                nc.sync.dma_start(
                    Xt[:, q * 2048:(q + 1) * 2048],
                    xt_d[:, q * 2048:(q + 1) * 2048],
                )
            for q in range(8):
                nc.sync.dma_start(
                    Wr[:, q * 2048:(q + 1) * 2048],
                    wr_d[:, q * 2048:(q + 1) * 2048],
                )

            # ---- s0 = X W / I : 32 accumulating full-depth matmuls ----
            q0 = pp.tile([P, 2048], F32, tag="quad")
            for c in range(32):
                nc.tensor.matmul(
                    q0[:, 0:KO],
                    Xt[:, c * 128:(c + 1) * 128],
                    Wr[:, c * KO:(c + 1) * KO],
                    start=(c == 0),
                    stop=(c == 31),
                )
            nc.scalar.activation(
                s0f[:], q0[:, 0:KO],
                mybir.ActivationFunctionType.Copy, scale=1.0 / I,
            )
            nc.scalar.mul(s0h[:], s0f[:], 1.0)

            # g = factor(s0)
            g32 = sm.tile([P, K], F32, tag="g32")
            factor(s0f[:], g32, "g")

            # ---- res production: (g, i, k8, o) order ----
            # c-quad cq, group g: 16 matmuls (band b, chunk 4cq+j), psum
            # cols b*512 + j*128 (bank=b avoids row-tile bank conflicts).
            for cq in range(8):
                for g in range(NG):
                    qt = pp.tile([P, 2048], F32, tag="quad")
                    for b in range(4):
                        r0 = b * 32
                        for j in range(4):
                            c = 4 * cq + j
                            nc.tensor.matmul(
                                qt[:, b * 512 + j * 128:b * 512 + (j + 1) * 128],
                                Xt[r0:r0 + 32, c * 128:(c + 1) * 128],
                                Wr[r0:r0 + 32, c * KO + g * 128:c * KO + (g + 1) * 128],
                                start=True,
                                stop=True,
                                tile_position=(r0, 0),
                            )
                    # evac: psum (b, j, ko) -> res cols base + j*512 + b*128
                    base = g * GN + cq * 2048
                    dst = (
                        res[:, base:base + 2048]
                        .rearrange("p (j b o) -> p j b o", j=4, b=4)
                        .transpose([0, 2, 1, 3])
                    )
                    src = qt[:].rearrange("p (b j o) -> p b j o", b=4, j=4)
                    with nc.allow_low_precision(reason="res fp16"):
                        nc.scalar.copy(dst, src)

        # ---- routing ---- (scratch pool opens after prep frees Xt/Wr)
        scr = ctx.enter_context(tc.tile_pool(name="scr", bufs=1))
        S1 = scr.tile([P, 16384], F16, tag="S1")
        S2 = scr.tile([P, 16384], F16, tag="S2")

        def half_round(v_h16, y_out16, nquart):
            """y = U^T (U v): uv+o-tree+q2, ut+i-tree, per group."""
            for g in range(NG):
                rg = res[:, g * GN:(g + 1) * GN]
                rv = rg.rearrange("p (i k o) -> p i k o", i=I, k=KC, o=O)
                vg = (
                    v_h16[:, g * KC * O:(g + 1) * KC * O]
                    .rearrange("p (k o) -> p k o", k=KC)
                )
                tmpv = S1[:].rearrange("p (i k o) -> p i k o", i=I, k=KC, o=O)
                # uv mul in nquart slices of i (overlap with production)
                istep = I // nquart
                for h in range(nquart):
                    i0, i1 = h * istep, (h + 1) * istep
                    nc.vector.tensor_mul(
                        tmpv[:, i0:i1],
                        rv[:, i0:i1],
                        vg.unsqueeze(1).broadcast_to([P, istep, KC, O]),
                    )
                # o-tree 16->8->4->2 (ik merged, contiguous)
                tv = S1[:].rearrange("p (ik o) -> p ik o", o=16)
                t8 = S2[:, 0:8192].rearrange("p (ik o) -> p ik o", o=8)
                nc.vector.tensor_add(t8, tv[:, :, 0:8], tv[:, :, 8:16])
                t4 = S1[:, 0:4096].rearrange("p (ik o) -> p ik o", o=4)
                nc.vector.tensor_add(t4, t8[:, :, 0:4], t8[:, :, 4:8])
                t2 = S1[:, 4096:6144].rearrange("p (ik o) -> p ik o", o=2)
                nc.vector.tensor_add(t2, t4[:, :, 0:2], t4[:, :, 2:4])
                # q2[ik, j] = t2[ik,0]+t2[ik,1]  (1x, dup'd output)
                q2 = S1[:, 6144:8192].rearrange("p (ik j) -> p ik j", j=2)
                nc.vector.tensor_add(
                    q2,
                    t2[:, :, 0:1].broadcast_to([P, I * KC, 2]),
                    t2[:, :, 1:2].broadcast_to([P, I * KC, 2]),
                )
                # ut mul: pair view (oh=8, ol=2)
                t2m = S2[:].rearrange("p (ik oh ol) -> p ik oh ol", oh=8, ol=2)
                rm = rg.rearrange("p (ik oh ol) -> p ik oh ol", oh=8, ol=2)
                nc.vector.tensor_mul(
                    t2m, rm, q2.unsqueeze(2).broadcast_to([P, I * KC, 8, 2])
                )
                # i-tree: 128 -> 1 over outer i
                spots = [
                    (S1, 0, 8192), (S2, 0, 4096), (S1, 8192, 2048),
                    (S2, 4096, 1024), (S1, 10240, 512), (S2, 5120, 256),
                ]
                cur, size = S2[:], I
                curv = cur.rearrange("p (i r) -> p i r", i=size)
                for lv in range(7):
                    half = size // 2
                    if lv < 6:
                        tb, off, ln = spots[lv]
                        nxt = tb[:, off:off + ln]
                    else:
                        nxt = y_out16[:, g * KC * O:(g + 1) * KC * O]
                    nv = nxt.rearrange("p (i r) -> p i r", i=half)
                    nc.vector.tensor_add(nv, curv[:, 0:half], curv[:, half:size])
                    cur, size, curv = nxt, half, nv

        with nc.allow_low_precision(reason="fp16 routing"):
            # round 1: y1 = A s0
            half_round(s0h, y1h, nquart=4)
            # f = factor(s1), s1 = s0 + g*y1  (overlaps round 2)
            # (s1 lives in outf's buffer; outf is only written at the end)
            outf = sm.tile([P, KO], F32, tag="outf")
            s1f = outf
            nc.vector.tensor_mul(
                s1f[:].rearrange("p (k o) -> p k o", k=K),
                y1h[:].rearrange("p (k o) -> p k o", k=K),
                bcast_o(g32),
            )
            nc.vector.tensor_add(s1f[:], s1f[:], s0f[:])
            # round 2: y2 = A y1
            half_round(y1h, y2h, nquart=1)

            f32_ = sm.tile([P, K], F32, tag="f32_")
            factor(s1f[:], f32_, "f")
            # s2 = s0 + (g+f)*y1 + (f*g)*y2
            gf = sm.tile([P, K], F32, tag="gf")
            nc.vector.tensor_add(gf[:], g32[:], f32_[:])
            fg = sm.tile([P, K], F32, tag="fg")
            nc.vector.tensor_mul(fg[:], f32_[:], g32[:])
            nc.vector.tensor_mul(
                s2f[:].rearrange("p (k o) -> p k o", k=K),
                y1h[:].rearrange("p (k o) -> p k o", k=K),
                bcast_o(gf),
            )
            nc.vector.tensor_add(s2f[:], s2f[:], s0f[:])
            nc.vector.tensor_mul(
                sqb[:].rearrange("p (k o) -> p k o", k=K),
                y2h[:].rearrange("p (k o) -> p k o", k=K),
                bcast_o(fg),
            )
            nc.vector.tensor_add(s2f[:], s2f[:], sqb[:])
            # out = factor(s2) * s2
            h32 = sm.tile([P, K], F32, tag="h32")
            factor(s2f[:], h32, "h")
            nc.vector.tensor_mul(
                outf[:].rearrange("p (k o) -> p k o", k=K),
                s2f[:].rearrange("p (k o) -> p k o", k=K),
                bcast_o(h32),
            )
        nc.sync.dma_start(out_d, outf[:])

    nc.compile()
    return nc


def _host_prep(x, W):
    """x: [B,R,C,I,D] f32; W: [K,I,D,O] f32 -> per-core Xt + shared W_r."""
    # Xt[(i%4)*32+d, (i//4)*128+p] = x[p, i, d]  (per core, d<16; pad to 32)
    xs = x.reshape(N_CORES, P, I, D)
    a = xs.transpose(0, 2, 3, 1).reshape(N_CORES, 32, 4, D, P)
    ap = np.zeros((N_CORES, 32, 4, D2, P), np.float32)
    ap[:, :, :, 0:D, :] = a
    xt = (
        ap.transpose(0, 2, 3, 1, 4)
        .reshape(N_CORES, 128, 32 * 128)
        .astype(np.float16)
    )
    # W_r[(i%4)*32+d, (i//4)*512 + k*16+o] = W[k, i, d, o]
    b = W.transpose(1, 2, 0, 3).reshape(32, 4, D, KO)
    bp = np.zeros((32, 4, D2, KO), np.float32)
    bp[:, :, 0:D, :] = b
    wr = bp.transpose(1, 2, 0, 3).reshape(128, 32 * KO).astype(np.float16)
    return xt, wr


def _get_program():
    global _PROGRAM
    if _PROGRAM is None:
        _PROGRAM = _build_program()
    return _PROGRAM


def kernel(**inputs):
    x = np.ascontiguousarray(np.asarray(inputs["inputs"], dtype=np.float32))
    W = np.ascontiguousarray(np.asarray(inputs["W"], dtype=np.float32))
    assert x.shape == (16, 8, 8, 128, 16) and W.shape == (32, 128, 16, 16)

    from concourse.bass_utils import run_bass_kernel_spmd

    nc = _get_program()
    xt, wr = _host_prep(x, W)
    in_maps = [
        {"xt": np.ascontiguousarray(xt[c]), "wr": wr} for c in range(N_CORES)
    ]
    r = run_bass_kernel_spmd(nc, in_maps, list(range(N_CORES)))
    outs = [r.results[c]["out"].reshape(2, 8, 8, K, O) for c in range(N_CORES)]
    return np.concatenate(outs, axis=0).astype(np.float32)
